# revision 32
# baseline (speedup 1.0000x reference)
"""Local 3x3 attention on 8 TRN2 NeuronCores — wire-optimized Bass/Tile kernel.

Problem: q,k,v [32, 128, 64, 64] f32; per pixel, attend over the 3x3
neighborhood (zero-padded) with softmax over the 9 logits; out [32,64,64,128].

The end-to-end wall time is dominated by the host<->device link (~45 MB/s,
mostly half-duplex), so the kernel minimizes wire bytes:

  * inputs ship as int9 (byte-plane encoded: an 8-bit lo plane plus a
    packed hi plane holding the top bit of eight pixels per byte) with a
    per-core dynamic scale -> 57 MB instead of 96 MB fp16;
  * the output ships as offset-int8 in the natural [pix, d] layout
    (16.8 MB instead of 33.5 MB fp16), normalized per (tile, partition)
    by the on-device abs-max of the numerator; a tiny [128, NT] f32
    scale tensor rides back alongside so the host can undo it (the
    softmax denominator cancels out of the quantization entirely);
  * masks / identity / denominator-correction constants are committed to
    the devices once and never re-sent;
  * packing runs per-core and each core's planes are device_put as soon
    as they're ready, so numpy pack time hides under the link streaming.

Device algorithm (per image, [128 d, 4096 pix] layout, pixel tiles of 128 =
two image rows):
  u8 planes are DMA'd and decoded on-device to integer-valued fp16
  (val = lo + (hi2<<8) - 512, exact in fp16).  v is decoded the same way
  then PE-transposed (identity matmul) into vT [pix, d] with a 1.0 ones
  column per 130-block so the AV matmul accumulates the softmax
  denominator for free.  Scores are computed transposed via PE matmuls
  contracting over d; ScalarE applies exp(s_act*x) out of PSUM where
  s_act = qk_scale/(sq*sk) arrives as a per-call [128,1] input; a 0/1
  band mask zeroes non-neighbor entries.  Out-of-image neighbors
  contribute exp(0)=1 to the reference denominator; a per-pixel constant
  corr tile (pre-divided by the int8 output fold G) adds that count.
  VectorE computes rec = recip(sum/G + corr/G) = G/denom, the
  per-partition abs-max mx of the AV numerator, ships mr = mx*rec, and
  the final fused multiply-add writes round(av*127.49/mx + 128) straight
  to uint8.  rel err ~1.2e-2 vs the 2e-2 gate.
"""

import numpy as np
from contextlib import ExitStack

import concourse.bass as bass
import concourse.tile as tile
from concourse import mybir

B, D, H, W = 32, 128, 64, 64
NCORES = 8
BL = B // NCORES          # images per core
NPIX = H * W              # 4096
NT = H // 2               # 32 two-row window tiles per image
SCALE = float(D) ** -0.5
QMAX = 255.49             # int9 half-range (values quantize to [1,511])
QOFF = 256.0              # int9 zero point
OMAX = 127.49             # int8 half-range
G = OMAX / QMAX           # denominator unit constant (any value works;
                          # the host folds it back out of the osc scales)
F16 = mybir.dt.float16
F32 = mybir.dt.float32
U8 = mybir.dt.uint8
EXP = mybir.ActivationFunctionType.Exp
COPY = mybir.ActivationFunctionType.Copy
ALU = mybir.AluOpType


def _host_consts():
    c = np.arange(128) % 64            # pixel column within its row
    hc = np.arange(64)                 # halo column
    band = (np.abs(hc[:, None] - c[None, :]) <= 1).astype(np.float16)  # [64,128]
    m_pair = np.concatenate([band, band], axis=0)                      # [128,128]
    m_first = band * (np.arange(128)[None, :] < 64)    # halo row 2t-1: r=0 only
    m_last = band * (np.arange(128)[None, :] >= 64)    # halo row 2t+2: r=1 only
    # single combined mask [128,256]: cols 0:128 = pair chunk, cols 128:256 =
    # singles chunk (partitions 0:64 = m_last at base 0, 64:128 = m_first at
    # base 64, matching the vT half-slices used in the AV matmuls).
    m_sing = np.concatenate([m_last, m_first], axis=0)
    m_all = np.concatenate([m_pair, m_sing] * 4, axis=1).astype(np.float16)
    # corr[p, t] = number of out-of-image neighbors for pixel p of tile t
    # (each contributes exp(0)=1 to the reference softmax denominator),
    # pre-divided by G to match the folded denominator units.
    r = np.arange(128) // 64
    vc = np.where((c == 0) | (c == 63), 2, 3)
    corr = np.zeros((128, NT), np.float32)
    for t in range(NT):
        vd = np.full(128, 3)
        if t == 0:
            vd = np.where(r == 0, 2, 3)
        if t == NT - 1:
            vd = np.where(r == 1, 2, 3)
        corr[:, t] = (9 - vd * vc) / G
    return m_all, corr


def _body(ctx, tc, qlo, qhi, klo, khi, vlo, vhi, sd, cr, mp, idn_d, od, osd,
          sim=False, bl=BL):
    nc = tc.nc

    consts = ctx.enter_context(tc.tile_pool(name="consts", bufs=1))
    planes = ctx.enter_context(tc.tile_pool(name="planes", bufs=2))
    bits = ctx.enter_context(tc.tile_pool(name="bits", bufs=2))
    lof_pool = ctx.enter_context(tc.tile_pool(name="lof", bufs=2))
    dec_pool = ctx.enter_context(tc.tile_pool(name="dec", bufs=4))
    vdec_pool = ctx.enter_context(tc.tile_pool(name="vdec", bufs=2))
    vt_pool = ctx.enter_context(tc.tile_pool(name="vt", bufs=1))
    ps_sc = ctx.enter_context(tc.tile_pool(name="ps_sc", bufs=2, space="PSUM"))
    ps_av = ctx.enter_context(tc.tile_pool(name="ps_av", bufs=3, space="PSUM"))
    ps_tr = ctx.enter_context(tc.tile_pool(name="ps_tr", bufs=1, space="PSUM"))
    at_pool = ctx.enter_context(tc.tile_pool(name="at", bufs=8))
    sm_pool = ctx.enter_context(tc.tile_pool(name="sm", bufs=8))
    out_pool = ctx.enter_context(tc.tile_pool(name="outp", bufs=4))

    # one-time constants (committed device-side across calls by the host)
    m_all4 = consts.tile([128, 1024], F16, tag="mall")
    nc.gpsimd.dma_start(m_all4[:], mp[:])
    corr = consts.tile([128, NT], F32, tag="corr")
    nc.gpsimd.dma_start(corr[:], cr[:])
    idn = consts.tile([128, 128], F16, tag="idn")
    nc.gpsimd.dma_start(idn[:], idn_d[:])
    scl = consts.tile([128, 1], F32, tag="scl")
    nc.gpsimd.dma_start(scl[:], sd[:])

    # vT double buffers: per-tile 130 cols = 128 d + ones col + pad.
    vt_all = [vt_pool.tile([128, NT * 130], F16, tag=f"vt{s}",
                           name=f"vt{s}") for s in range(2)]
    for s in range(2):
        ones = vt_all[s][:].rearrange('p (t c) -> p t c', c=130)[:, :, 128:130]
        nc.vector.memset(ones, 0.0)
        nc.vector.memset(
            vt_all[s][:].rearrange('p (t c) -> p t c', c=130)[:, :, 128:129],
            1.0)

    def decode(lo_t, hi_t, dst, lof):
        # u9 planes -> integer-valued fp16 in [-256, 255]
        nc.scalar.activation(lof[:], lo_t[:], COPY, bias=-QOFF)
        for j in range(8):
            b = bits.tile([128, 512], U8, tag=f"b{j}")
            nc.vector.tensor_scalar(b[:], hi_t[:], j, 1,
                                    ALU.logical_shift_right, ALU.bitwise_and)
            nc.vector.scalar_tensor_tensor(
                dst[:, 512 * j:512 * (j + 1)], b[:], 256.0,
                lof[:, 512 * j:512 * (j + 1)], ALU.mult, ALU.add)

    for i in range(bl):
        s = i % 2
        ql = planes.tile([128, NPIX], U8, tag="ql")
        qh = planes.tile([128, NPIX // 8], U8, tag="qh")
        kl = planes.tile([128, NPIX], U8, tag="kl")
        kh = planes.tile([128, NPIX // 8], U8, tag="kh")
        vl = planes.tile([128, NPIX], U8, tag="vl")
        vh = planes.tile([128, NPIX // 8], U8, tag="vh")
        nc.sync.dma_start(ql[:], qlo[i][:, 0:NPIX])
        nc.sync.dma_start(qh[:], qlo[i][:, NPIX:])
        nc.sync.dma_start(kl[:], klo[i][:, 0:NPIX])
        nc.sync.dma_start(kh[:], klo[i][:, NPIX:])
        nc.sync.dma_start(vl[:], vlo[i][:, 0:NPIX])
        nc.sync.dma_start(vh[:], vlo[i][:, NPIX:])

        q_t = dec_pool.tile([128, NPIX], F16, tag="q")
        k_t = dec_pool.tile([128, NPIX], F16, tag="k")
        v_t = vdec_pool.tile([128, NPIX], F16, tag="v")
        decode(ql, qh, q_t,
               lof_pool.tile([128, NPIX], F16, tag="lq", name="lq"))
        decode(kl, kh, k_t,
               lof_pool.tile([128, NPIX], F16, tag="lk", name="lk"))
        decode(vl, vh, v_t,
               lof_pool.tile([128, NPIX], F16, tag="lv", name="lv"))

        # vT via PE transpose, 8 tiles per PSUM bank, 4 strided copies out
        vt_r = vt_all[s][:].rearrange('p (t c) -> p t c', c=130)
        for g8 in range(NT // 8):
            pt = ps_tr.tile([128, 1024], F16, tag="pt")
            for j in range(8):
                t = 8 * g8 + j
                nc.tensor.transpose(pt[:, 128 * j:128 * (j + 1)],
                                    v_t[:, 128 * t:128 * (t + 1)], idn[:])
            nc.scalar.copy(vt_r[:, 8 * g8:8 * (g8 + 1), 0:128],
                           pt[:].rearrange('p (t c) -> p t c', c=128))

        # per-image staging for the per-(tile,partition) output scales
        mr_t = sm_pool.tile([128, NT], F32, tag="mr", name="mr")

        # one iteration per QUAD of window tiles for scores/exp/mask;
        # AV + epilogue run per pair inside.
        for w in range(NT // 4):
            ts4 = [4 * w + j for j in range(4)]
            sc_a = ps_sc.tile([128, 512], F32, tag="sca")
            sc_b = ps_sc.tile([128, 512], F32, tag="scb")
            for h, t in enumerate(ts4):
                sc = sc_a if h < 2 else sc_b
                qs = q_t[:, 128 * t:128 * (t + 1)]
                o = 256 * (h % 2)
                nc.tensor.matmul(sc[:, o:o + 128],
                                 lhsT=k_t[:, 128 * t:128 * (t + 1)],
                                 rhs=qs, start=True, stop=True)
                if t < NT - 1:
                    nc.tensor.matmul(sc[0:64, o + 128:o + 256],
                                     lhsT=k_t[:, 64 * (2 * t + 2):64 * (2 * t + 3)],
                                     rhs=qs, start=True, stop=True)
                elif sim:
                    nc.vector.memset(sc[0:64, o + 128:o + 256], 0.0)
                if t > 0:
                    nc.tensor.matmul(sc[64:128, o + 128:o + 256],
                                     lhsT=k_t[:, 64 * (2 * t - 1):64 * 2 * t],
                                     rhs=qs, start=True, stop=True)
                elif sim:
                    nc.vector.memset(sc[64:128, o + 128:o + 256], 0.0)
            # exp with the dynamic per-call scale, then band mask
            at = at_pool.tile([128, 1024], F16, tag="at")
            nc.scalar.activation(at[:, 0:512], sc_a[:], EXP, scale=scl[:, 0:1])
            nc.scalar.activation(at[:, 512:1024], sc_b[:], EXP,
                                 scale=scl[:, 0:1])
            nc.gpsimd.tensor_mul(at[:, 0:512], at[:, 0:512], m_all4[:, 0:512])
            nc.gpsimd.tensor_mul(at[:, 512:1024], at[:, 512:1024],
                                 m_all4[:, 512:1024])
            o_t = out_pool.tile([128, 512], U8, tag="o")
            for g in range(2):  # two pairs in the quad
                t0 = ts4[2 * g]
                for h2 in range(2):
                    t = t0 + h2
                    o = 256 * (2 * g + h2)
                    av2 = ps_av.tile([128, 132], F32, tag="av2")
                    avs = av2[:, 0:129]
                    mm = []
                    if t > 0:
                        mm.append((at[64:128, o + 128:o + 256],
                                   vt_all[s][64:128,
                                             130 * (t - 1):130 * (t - 1) + 129]))
                    mm.append((at[:, o:o + 128],
                               vt_all[s][:, 130 * t:130 * t + 129]))
                    if t < NT - 1:
                        mm.append((at[0:64, o + 128:o + 256],
                                   vt_all[s][0:64,
                                             130 * (t + 1):130 * (t + 1) + 129]))
                    for j, (a, vv) in enumerate(mm):
                        nc.tensor.matmul(avs, lhsT=a, rhs=vv,
                                         start=(j == 0), stop=(j == len(mm) - 1))
                    # dnm = sum(exp)/G + corr/G ; rec = G/denom
                    dnm = sm_pool.tile([128, 1], F32, tag="dnm")
                    nc.vector.scalar_tensor_tensor(dnm[:], av2[:, 128:129],
                                                   1.0 / G, corr[:, t:t + 1],
                                                   ALU.mult, ALU.add)
                    rec = sm_pool.tile([128, 1], F32, tag="rec")
                    nc.vector.reciprocal(rec[:], dnm[:])
                    # per-partition abs-max of the numerator: the u8 encode
                    # normalizes by it (denominator cancels); the host gets
                    # mr = mx*rec to undo both.
                    mx = sm_pool.tile([128, 1], F32, tag="mx")
                    nc.vector.tensor_reduce(mx[:], av2[:, 0:128],
                                            mybir.AxisListType.X, ALU.max,
                                            apply_absolute_value=True)
                    nc.vector.tensor_scalar_max(mx[:], mx[:], 1e-30)
                    nc.vector.tensor_mul(mr_t[:, t:t + 1], mx[:], rec[:])
                    rmx = sm_pool.tile([128, 1], F32, tag="rmx")
                    nc.vector.reciprocal(rmx[:], mx[:])
                    nc.vector.tensor_scalar_mul(rmx[:], rmx[:], OMAX)
                    gslot = 2 * g + h2
                    # HW f32->u8 conversion rounds (CoreSim truncates);
                    # +128.0 keeps the decode unbiased on HW
                    nc.vector.tensor_scalar(
                        o_t[:, 128 * gslot:128 * (gslot + 1)],
                        av2[:, 0:128], rmx[:], 128.0, ALU.mult, ALU.add)
            # natural [pix, d] layout: pix = 512w + 128*slot + p
            dd = od[i, 512 * w:512 * (w + 1), :].rearrange(
                '(g p) d -> p g d', g=4, p=128)
            nc.sync.dma_start(dd, o_t[:].rearrange('p (g x) -> p g x', g=4))
        nc.sync.dma_start(osd[i], mr_t[:])


def _legalize_dma_waits(nc):
    """walrus's PSEUDO_DMA_DIRECT2D codegen accepts at most one sync-wait
    per DMA instruction, but Tile emits 2-3 (own-lane ring wait + real
    deps).  Hoist every DMA wait onto same-engine NoOps directly before the
    DMA — engine program order makes this equivalent."""
    keep = (mybir.InstUnconditionalBranch, mybir.InstCompareAndBranch,
            mybir.InstHalt)
    f = nc.m.functions[0]
    for bb in f.blocks:
        new = []
        changed = False
        for inst in bb.instructions:
            si = inst.sync_info
            if (si is not None and len(si.on_wait) > 1
                    and not isinstance(inst, keep)):
                for wi, w in enumerate(si.on_wait[1:]):
                    new.append(mybir.InstNoOp(
                        name=f"{inst.name}-w{wi}",
                        engine=inst.engine,
                        sync_info=mybir.SyncInfo(on_wait=[w], on_update=[]),
                        bass_nofuse=True,
                    ))
                inst.sync_info = mybir.SyncInfo(
                    on_wait=[si.on_wait[0]], on_update=list(si.on_update))
                changed = True
            new.append(inst)
        if changed:
            bb.instructions = new


def _build(legalize=True, sim=False, bl=BL):
    nc = bass.Bass()
    wb = NPIX + NPIX // 8    # combined lo|hi plane width
    qlo = nc.declare_dram_parameter("qb", [bl, 128, wb], U8, isOutput=False)
    qhi = None
    klo = nc.declare_dram_parameter("kb", [bl, 128, wb], U8, isOutput=False)
    khi = None
    vlo = nc.declare_dram_parameter("vb", [bl, 128, wb], U8, isOutput=False)
    vhi = None
    sd = nc.declare_dram_parameter("scl", [128, 1], F32, isOutput=False)
    cr = nc.declare_dram_parameter("corr", [128, NT], F32, isOutput=False)
    mp = nc.declare_dram_parameter("mall", [128, 1024], F16, isOutput=False)
    idn = nc.declare_dram_parameter("idn", [128, 128], F16, isOutput=False)
    od = nc.declare_dram_parameter("out", [bl, NPIX, D], U8, isOutput=True)
    osd = nc.declare_dram_parameter("osc", [bl, 128, NT], F32, isOutput=True)
    with tile.TileContext(nc) as tc:
        with ExitStack() as ctx:
            _body(ctx, tc, qlo, qhi, klo, khi, vlo, vhi, sd, cr, mp, idn,
                  od, osd, sim=sim, bl=bl)
    if legalize:
        _legalize_dma_waits(nc)
    return nc


_CACHE = {}


def _get_nc():
    if "nc" not in _CACHE:
        _CACHE["nc"] = _build()
    return _CACHE["nc"]


_M_ALL, _CORR = _host_consts()
_IDN = np.eye(128, dtype=np.float16)


_PZ = np.empty((BL, 128, NPIX), np.float32)
_PB = np.empty((BL, 128, 8, NPIX // 8), bool)


def _pack9(x, s):
    """f32 [4,128,H,W] slice -> one [4,128,4608] u8 wire buffer.

    u9 = floor(x*s + 256.5) in [1,511]; cols 0:4096 hold u9 & 255 (the
    f32->u8 cast truncates then wraps mod 256), cols 4096:4608 pack the
    top bit of pixels j, j+512, ..., j+3584 into byte j.  Scratch buffers
    are module-level; the returned buffer is fresh (it is handed to async
    device_put).
    """
    np.multiply(x.reshape(-1, 128, NPIX), np.float32(s), out=_PZ)
    np.add(_PZ, np.float32(QOFF + 0.5), out=_PZ)
    buf = np.empty((BL, 128, NPIX + NPIX // 8), np.uint8)
    np.copyto(buf[:, :, :NPIX], _PZ, casting='unsafe')
    np.greater_equal(_PZ.reshape(-1, 128, 8, NPIX // 8),
                     np.float32(QOFF), out=_PB)
    bv = _PB.view(np.uint8)
    hb = buf[:, :, NPIX:]
    np.copyto(hb, bv[:, :, 0])
    for j in range(1, 8):
        np.left_shift(bv[:, :, j], j, out=bv[:, :, j])
        np.bitwise_or(hb, bv[:, :, j], out=hb)
    return buf


def _absmax(x):
    return float(max(x.max(), -float(x.min()), 1e-30))


def _get_runner():
    """Cached PJRT runner: one shard_map jit over 8 cores; per-call inputs
    arrive as already-committed per-device buffers assembled into global
    arrays (so transfers stream while later cores still pack)."""
    if "run" in _CACHE:
        return _CACHE["run"]
    import jax
    import jax.numpy as jnp
    from jax.sharding import Mesh, PartitionSpec, NamedSharding
    from concourse import bass2jax, mybir as mb

    nc = _get_nc()
    bass2jax.install_neuronx_cc_hook()

    part_name = nc.partition_id_tensor.name if nc.partition_id_tensor else None
    in_names, out_names, out_avals = [], [], []
    for alloc in nc.m.functions[0].allocations:
        if not isinstance(alloc, mb.MemoryLocationSet):
            continue
        name = alloc.memorylocations[0].name
        if alloc.kind == "ExternalInput":
            if name != part_name:
                in_names.append(name)
        elif alloc.kind == "ExternalOutput":
            out_names.append(name)
            out_avals.append(jax.core.ShapedArray(
                tuple(alloc.tensor_shape), mb.dt.np(alloc.dtype)))
    n_params = len(in_names)
    all_in_names = in_names + out_names
    if part_name is not None:
        all_in_names = all_in_names + [part_name]

    def _body_fn(*args):
        operands = list(args)
        if part_name is not None:
            operands.append(bass2jax.partition_id_tensor())
        outs = bass2jax._bass_exec_p.bind(
            *operands,
            out_avals=tuple(out_avals),
            in_names=tuple(all_in_names),
            out_names=tuple(out_names),
            lowering_input_output_aliases=(),
            sim_require_finite=True,
            sim_require_nnan=True,
            nc=nc,
        )
        return tuple(outs)

    devices = jax.devices()[:NCORES]
    mesh = Mesh(np.asarray(devices), ("core",))
    n_outs = len(out_names)
    in_specs = (PartitionSpec("core"),) * (n_params + n_outs)
    out_specs = (PartitionSpec("core"),) * n_outs
    from jax.experimental.shard_map import shard_map
    sharded = jax.jit(
        shard_map(_body_fn, mesh=mesh, in_specs=in_specs, out_specs=out_specs,
                  check_rep=False),
        donate_argnums=tuple(range(n_params, n_params + n_outs)),
        keep_unused=True,
    )
    sharding = NamedSharding(mesh, PartitionSpec("core"))
    zero_fns = [
        jax.jit(
            (lambda shp, dt: (lambda: jnp.zeros(shp, dt)))(
                (NCORES * av.shape[0], *av.shape[1:]), av.dtype),
            out_shardings=sharding)
        for av in out_avals
    ]

    # one-time committed constants
    dconsts = {
        "corr": jax.device_put(np.tile(_CORR, (NCORES, 1)), sharding),
        "mall": jax.device_put(np.tile(_M_ALL, (NCORES, 1)), sharding),
        "idn": jax.device_put(np.tile(_IDN, (NCORES, 1)), sharding),
    }
    for a in dconsts.values():
        a.block_until_ready()

    wb = NPIX + NPIX // 8
    gshapes = {
        "qb": (B, 128, wb), "kb": (B, 128, wb), "vb": (B, 128, wb),
    }

    def run(percore, gfeeds):
        # percore: dict name -> list of 8 per-device committed jax arrays;
        # gfeeds: dict name -> already-global committed jax arrays
        feed = dict(dconsts)
        feed.update(gfeeds)
        for name, bufs in percore.items():
            feed[name] = jax.make_array_from_single_device_arrays(
                gshapes[name], sharding, bufs)
        # donation buffers are prefetched at the end of the previous call
        zeros = _CACHE.pop("zeros_next", None) or [zf() for zf in zero_fns]
        outs = sharded(*[feed[n] for n in in_names], *zeros)
        for o in outs:
            o.copy_to_host_async()
        _CACHE["zeros_next"] = [zf() for zf in zero_fns]
        return {name: outs[i] for i, name in enumerate(out_names)}

    _CACHE["run"] = run
    _CACHE["devices"] = devices
    _CACHE["sharding"] = sharding
    return run


def _get_putter():
    if "put" not in _CACHE:
        from concurrent.futures import ThreadPoolExecutor
        _CACHE["put"] = ThreadPoolExecutor(max_workers=2)
    return _CACHE["put"]


def kernel(q, k, v):
    import jax
    run = _get_runner()          # ensures devices/consts ready
    devices = _CACHE["devices"]
    putter = _get_putter()
    q = np.asarray(q).reshape(B, D, H, W)
    k = np.asarray(k).reshape(B, D, H, W)
    v = np.asarray(v).reshape(B, D, H, W)

    futs = {n: [] for n in ("qb", "kb", "vb")}
    scl_h = np.empty((NCORES * 128, 1), np.float32)
    mv = []
    for c in range(NCORES):
        dev = devices[c]
        sl = slice(BL * c, BL * (c + 1))
        mq = _absmax(q[sl]); sq = QMAX / mq
        futs["qb"].append(putter.submit(jax.device_put, _pack9(q[sl], sq),
                                        dev))
        mk = _absmax(k[sl]); sk = QMAX / mk
        futs["kb"].append(putter.submit(jax.device_put, _pack9(k[sl], sk),
                                        dev))
        mvx = _absmax(v[sl]); sv = QMAX / mvx
        futs["vb"].append(putter.submit(jax.device_put, _pack9(v[sl], sv),
                                        dev))
        mv.append(mvx)
        scl_h[128 * c:128 * (c + 1)] = SCALE / (sq * sk)
    scl_f = putter.submit(jax.device_put, scl_h, _CACHE["sharding"])

    percore = {n: [f.result() for f in fs] for n, fs in futs.items()}
    outs = run(percore, {"scl": scl_f.result()})
    out_g = outs["out"]           # [B, NPIX, D] u8, 8-core sharded
    osc_g = outs["osc"]           # [B, 128, NT] f32, mr = mx*G/denom

    res = np.empty((B, NPIX, D), np.float32)
    shards = {int(s.index[0].start) // BL: s
              for s in out_g.addressable_shards}
    oshards = {int(s.index[0].start) // BL: s
               for s in osc_g.addressable_shards}
    for c in range(NCORES):
        oc = np.asarray(shards[c].data)    # [BL, NPIX, D] u8
        mr = np.asarray(oshards[c].data)   # [BL, 128, NT] f32
        # out = (u8-128) * mr * Mv / (OMAX * G * QMAX), scale per (i,t,p)
        wgt = mr.transpose(0, 2, 1) * np.float32(mv[c] / (OMAX * G * QMAX))
        rs = res[BL * c:BL * (c + 1)]
        np.subtract(oc.astype(np.float32), np.float32(128.0), out=rs)
        np.multiply(rs.reshape(BL, NT, 128, D), wgt[:, :, :, None], out=rs.reshape(BL, NT, 128, D))
    return res.reshape(B, H, W, D)


def _profile(inputs):
    """Run once under the axon NTFF profiling hook; returns max-core HW
    exec time in ns (None if the hook is unavailable). For test.py only."""
    from concourse.bass_utils import run_bass_kernel_spmd
    nc = _get_nc()
    q = np.asarray(inputs["q"]).reshape(B, D, H, W)
    k = np.asarray(inputs["k"]).reshape(B, D, H, W)
    v = np.asarray(inputs["v"]).reshape(B, D, H, W)
    in_maps = []
    for c in range(NCORES):
        sl = slice(BL * c, BL * (c + 1))
        mq = _absmax(q[sl]); sq = QMAX / mq
        mk = _absmax(k[sl]); sk = QMAX / mk
        mvx = _absmax(v[sl]); sv = QMAX / mvx
        in_maps.append({
            "qb": _pack9(q[sl], sq), "kb": _pack9(k[sl], sk),
            "vb": _pack9(v[sl], sv),
            "scl": np.full((128, 1), SCALE / (sq * sk), np.float32),
            "corr": _CORR, "mall": _M_ALL, "idn": _IDN,
        })
    br = run_bass_kernel_spmd(nc, in_maps, core_ids=list(range(NCORES)),
                              trace=True)
    res = {
        "exec_time_ns": br.exec_time_ns,
        "mean_exec_time_ns": br.mean_exec_time_ns,
        "trace": br.instructions_and_trace,
    }
    _CACHE["last_profile"] = res
    if br.instructions_and_trace is not None:
        print("trace path:", br.instructions_and_trace[1])
    return br.exec_time_ns


# revision 33
# speedup vs baseline: 1.0289x; 1.0289x over previous
"""Local 3x3 attention on 8 TRN2 NeuronCores — wire-optimized Bass/Tile kernel.

Problem: q,k,v [32, 128, 64, 64] f32; per pixel, attend over the 3x3
neighborhood (zero-padded) with softmax over the 9 logits; out [32,64,64,128].

The end-to-end wall time is dominated by the host<->device link (~45 MB/s,
mostly half-duplex), so the kernel minimizes wire bytes:

  * inputs ship as int9 (byte-plane encoded: an 8-bit lo plane plus a
    packed hi plane holding the top bit of eight pixels per byte) with a
    per-core dynamic scale -> 57 MB instead of 96 MB fp16;
  * the output ships as offset-int8 in the natural [pix, d] layout
    (16.8 MB instead of 33.5 MB fp16), normalized per (tile, partition)
    by the on-device abs-max of the numerator; a tiny [128, NT] f32
    scale tensor rides back alongside so the host can undo it (the
    softmax denominator cancels out of the quantization entirely);
  * masks / identity / denominator-correction constants are committed to
    the devices once and never re-sent;
  * packing runs per-core and each core's planes are device_put as soon
    as they're ready, so numpy pack time hides under the link streaming.

Device algorithm (per image, [128 d, 4096 pix] layout, pixel tiles of 128 =
two image rows):
  u8 planes are DMA'd and decoded on-device to integer-valued fp16
  (val = lo + (hi2<<8) - 512, exact in fp16).  v is decoded the same way
  then PE-transposed (identity matmul) into vT [pix, d] with a 1.0 ones
  column per 130-block so the AV matmul accumulates the softmax
  denominator for free.  Scores are computed transposed via PE matmuls
  contracting over d; ScalarE applies exp(s_act*x) out of PSUM where
  s_act = qk_scale/(sq*sk) arrives as a per-call [128,1] input; a 0/1
  band mask zeroes non-neighbor entries.  Out-of-image neighbors
  contribute exp(0)=1 to the reference denominator; a per-pixel constant
  corr tile (pre-divided by the int8 output fold G) adds that count.
  VectorE computes rec = recip(sum/G + corr/G) = G/denom, the
  per-partition abs-max mx of the AV numerator, ships mr = mx*rec, and
  the final fused multiply-add writes round(av*127.49/mx + 128) straight
  to uint8.  rel err ~1.2e-2 vs the 2e-2 gate.
"""

import numpy as np
from contextlib import ExitStack

import concourse.bass as bass
import concourse.tile as tile
from concourse import mybir

B, D, H, W = 32, 128, 64, 64
NCORES = 8
BL = B // NCORES          # images per core
NPIX = H * W              # 4096
NT = H // 2               # 32 two-row window tiles per image
SCALE = float(D) ** -0.5
QMAX = 255.49             # int9 half-range (values quantize to [1,511])
QOFF = 256.0              # int9 zero point
OMAX = 127.49             # int8 half-range
G = OMAX / QMAX           # denominator unit constant (any value works;
                          # the host folds it back out of the osc scales)
F16 = mybir.dt.float16
F32 = mybir.dt.float32
U8 = mybir.dt.uint8
EXP = mybir.ActivationFunctionType.Exp
COPY = mybir.ActivationFunctionType.Copy
ALU = mybir.AluOpType


def _host_consts():
    c = np.arange(128) % 64            # pixel column within its row
    hc = np.arange(64)                 # halo column
    band = (np.abs(hc[:, None] - c[None, :]) <= 1).astype(np.float16)  # [64,128]
    m_pair = np.concatenate([band, band], axis=0)                      # [128,128]
    m_first = band * (np.arange(128)[None, :] < 64)    # halo row 2t-1: r=0 only
    m_last = band * (np.arange(128)[None, :] >= 64)    # halo row 2t+2: r=1 only
    # single combined mask [128,256]: cols 0:128 = pair chunk, cols 128:256 =
    # singles chunk (partitions 0:64 = m_last at base 0, 64:128 = m_first at
    # base 64, matching the vT half-slices used in the AV matmuls).
    m_sing = np.concatenate([m_last, m_first], axis=0)
    m_all = np.concatenate([m_pair, m_sing] * 4, axis=1).astype(np.float16)
    # corr[p, t] = number of out-of-image neighbors for pixel p of tile t
    # (each contributes exp(0)=1 to the reference softmax denominator),
    # pre-divided by G to match the folded denominator units.
    r = np.arange(128) // 64
    vc = np.where((c == 0) | (c == 63), 2, 3)
    corr = np.zeros((128, NT), np.float32)
    for t in range(NT):
        vd = np.full(128, 3)
        if t == 0:
            vd = np.where(r == 0, 2, 3)
        if t == NT - 1:
            vd = np.where(r == 1, 2, 3)
        corr[:, t] = (9 - vd * vc) / G
    return m_all, corr


def _body(ctx, tc, qlo, qhi, klo, khi, vlo, vhi, sd, cr, mp, idn_d, od, osd,
          sim=False, bl=BL):
    nc = tc.nc

    consts = ctx.enter_context(tc.tile_pool(name="consts", bufs=1))
    planes = ctx.enter_context(tc.tile_pool(name="planes", bufs=2))
    bits = ctx.enter_context(tc.tile_pool(name="bits", bufs=2))
    lof_pool = ctx.enter_context(tc.tile_pool(name="lof", bufs=2))
    dec_pool = ctx.enter_context(tc.tile_pool(name="dec", bufs=4))
    vdec_pool = ctx.enter_context(tc.tile_pool(name="vdec", bufs=2))
    vt_pool = ctx.enter_context(tc.tile_pool(name="vt", bufs=1))
    ps_sc = ctx.enter_context(tc.tile_pool(name="ps_sc", bufs=2, space="PSUM"))
    ps_av = ctx.enter_context(tc.tile_pool(name="ps_av", bufs=3, space="PSUM"))
    ps_tr = ctx.enter_context(tc.tile_pool(name="ps_tr", bufs=1, space="PSUM"))
    at_pool = ctx.enter_context(tc.tile_pool(name="at", bufs=8))
    sm_pool = ctx.enter_context(tc.tile_pool(name="sm", bufs=8))
    out_pool = ctx.enter_context(tc.tile_pool(name="outp", bufs=4))

    # one-time constants (committed device-side across calls by the host)
    m_all4 = consts.tile([128, 1024], F16, tag="mall")
    nc.gpsimd.dma_start(m_all4[:], mp[:])
    corr = consts.tile([128, NT], F32, tag="corr")
    nc.gpsimd.dma_start(corr[:], cr[:])
    idn = consts.tile([128, 128], F16, tag="idn")
    nc.gpsimd.dma_start(idn[:], idn_d[:])
    scl = consts.tile([128, 1], F32, tag="scl")
    nc.gpsimd.dma_start(scl[:], sd[:])

    # vT double buffers: per-tile 130 cols = 128 d + ones col + pad.
    vt_all = [vt_pool.tile([128, NT * 130], F16, tag=f"vt{s}",
                           name=f"vt{s}") for s in range(2)]
    for s in range(2):
        ones = vt_all[s][:].rearrange('p (t c) -> p t c', c=130)[:, :, 128:130]
        nc.vector.memset(ones, 0.0)
        nc.vector.memset(
            vt_all[s][:].rearrange('p (t c) -> p t c', c=130)[:, :, 128:129],
            1.0)

    def decode(lo_t, hi_t, dst, lof):
        # u9 planes -> integer-valued fp16 in [-256, 255]
        nc.scalar.activation(lof[:], lo_t[:], COPY, bias=-QOFF)
        for j in range(8):
            b = bits.tile([128, 512], U8, tag=f"b{j}")
            nc.vector.tensor_scalar(b[:], hi_t[:], j, 1,
                                    ALU.logical_shift_right, ALU.bitwise_and)
            nc.vector.scalar_tensor_tensor(
                dst[:, 512 * j:512 * (j + 1)], b[:], 256.0,
                lof[:, 512 * j:512 * (j + 1)], ALU.mult, ALU.add)

    for i in range(bl):
        s = i % 2
        ql = planes.tile([128, NPIX], U8, tag="ql")
        qh = planes.tile([128, NPIX // 8], U8, tag="qh")
        kl = planes.tile([128, NPIX], U8, tag="kl")
        kh = planes.tile([128, NPIX // 8], U8, tag="kh")
        vl = planes.tile([128, NPIX], U8, tag="vl")
        vh = planes.tile([128, NPIX // 8], U8, tag="vh")
        nc.sync.dma_start(ql[:], qlo[i][:, 0:NPIX])
        nc.sync.dma_start(qh[:], qlo[i][:, NPIX:])
        nc.sync.dma_start(kl[:], klo[i][:, 0:NPIX])
        nc.sync.dma_start(kh[:], klo[i][:, NPIX:])
        nc.sync.dma_start(vl[:], vlo[i][:, 0:NPIX])
        nc.sync.dma_start(vh[:], vlo[i][:, NPIX:])

        q_t = dec_pool.tile([128, NPIX], F16, tag="q")
        k_t = dec_pool.tile([128, NPIX], F16, tag="k")
        v_t = vdec_pool.tile([128, NPIX], F16, tag="v")
        decode(ql, qh, q_t,
               lof_pool.tile([128, NPIX], F16, tag="lq", name="lq"))
        decode(kl, kh, k_t,
               lof_pool.tile([128, NPIX], F16, tag="lk", name="lk"))
        decode(vl, vh, v_t,
               lof_pool.tile([128, NPIX], F16, tag="lv", name="lv"))

        # vT via PE transpose, 8 tiles per PSUM bank, 4 strided copies out
        vt_r = vt_all[s][:].rearrange('p (t c) -> p t c', c=130)
        for g8 in range(NT // 8):
            pt = ps_tr.tile([128, 1024], F16, tag="pt")
            for j in range(8):
                t = 8 * g8 + j
                nc.tensor.transpose(pt[:, 128 * j:128 * (j + 1)],
                                    v_t[:, 128 * t:128 * (t + 1)], idn[:])
            nc.scalar.copy(vt_r[:, 8 * g8:8 * (g8 + 1), 0:128],
                           pt[:].rearrange('p (t c) -> p t c', c=128))

        # per-image staging for the per-(tile,partition) output scales
        mr_t = sm_pool.tile([128, NT], F32, tag="mr", name="mr")

        # one iteration per QUAD of window tiles for scores/exp/mask;
        # AV + epilogue run per pair inside.
        for w in range(NT // 4):
            ts4 = [4 * w + j for j in range(4)]
            sc_a = ps_sc.tile([128, 512], F32, tag="sca")
            sc_b = ps_sc.tile([128, 512], F32, tag="scb")
            for h, t in enumerate(ts4):
                sc = sc_a if h < 2 else sc_b
                qs = q_t[:, 128 * t:128 * (t + 1)]
                o = 256 * (h % 2)
                nc.tensor.matmul(sc[:, o:o + 128],
                                 lhsT=k_t[:, 128 * t:128 * (t + 1)],
                                 rhs=qs, start=True, stop=True)
                if t < NT - 1:
                    nc.tensor.matmul(sc[0:64, o + 128:o + 256],
                                     lhsT=k_t[:, 64 * (2 * t + 2):64 * (2 * t + 3)],
                                     rhs=qs, start=True, stop=True)
                elif sim:
                    nc.vector.memset(sc[0:64, o + 128:o + 256], 0.0)
                if t > 0:
                    nc.tensor.matmul(sc[64:128, o + 128:o + 256],
                                     lhsT=k_t[:, 64 * (2 * t - 1):64 * 2 * t],
                                     rhs=qs, start=True, stop=True)
                elif sim:
                    nc.vector.memset(sc[64:128, o + 128:o + 256], 0.0)
            # exp with the dynamic per-call scale, then band mask
            at = at_pool.tile([128, 1024], F16, tag="at")
            nc.scalar.activation(at[:, 0:512], sc_a[:], EXP, scale=scl[:, 0:1])
            nc.scalar.activation(at[:, 512:1024], sc_b[:], EXP,
                                 scale=scl[:, 0:1])
            nc.gpsimd.tensor_mul(at[:, 0:512], at[:, 0:512], m_all4[:, 0:512])
            nc.gpsimd.tensor_mul(at[:, 512:1024], at[:, 512:1024],
                                 m_all4[:, 512:1024])
            o_t = out_pool.tile([128, 512], U8, tag="o")
            for g in range(2):  # two pairs in the quad
                t0 = ts4[2 * g]
                for h2 in range(2):
                    t = t0 + h2
                    o = 256 * (2 * g + h2)
                    av2 = ps_av.tile([128, 132], F32, tag="av2")
                    avs = av2[:, 0:129]
                    mm = []
                    if t > 0:
                        mm.append((at[64:128, o + 128:o + 256],
                                   vt_all[s][64:128,
                                             130 * (t - 1):130 * (t - 1) + 129]))
                    mm.append((at[:, o:o + 128],
                               vt_all[s][:, 130 * t:130 * t + 129]))
                    if t < NT - 1:
                        mm.append((at[0:64, o + 128:o + 256],
                                   vt_all[s][0:64,
                                             130 * (t + 1):130 * (t + 1) + 129]))
                    for j, (a, vv) in enumerate(mm):
                        nc.tensor.matmul(avs, lhsT=a, rhs=vv,
                                         start=(j == 0), stop=(j == len(mm) - 1))
                    # dnm = sum(exp)/G + corr/G ; rec = G/denom
                    dnm = sm_pool.tile([128, 1], F32, tag="dnm")
                    nc.vector.scalar_tensor_tensor(dnm[:], av2[:, 128:129],
                                                   1.0 / G, corr[:, t:t + 1],
                                                   ALU.mult, ALU.add)
                    rec = sm_pool.tile([128, 1], F32, tag="rec")
                    nc.vector.reciprocal(rec[:], dnm[:])
                    # per-partition abs-max of the numerator: the u8 encode
                    # normalizes by it (denominator cancels); the host gets
                    # mr = mx*rec to undo both.
                    mx = sm_pool.tile([128, 1], F32, tag="mx")
                    nc.vector.tensor_reduce(mx[:], av2[:, 0:128],
                                            mybir.AxisListType.X, ALU.max,
                                            apply_absolute_value=True)
                    nc.vector.tensor_scalar_max(mx[:], mx[:], 1e-30)
                    nc.vector.tensor_mul(mr_t[:, t:t + 1], mx[:], rec[:])
                    rmx = sm_pool.tile([128, 1], F32, tag="rmx")
                    nc.vector.reciprocal(rmx[:], mx[:])
                    nc.vector.tensor_scalar_mul(rmx[:], rmx[:], OMAX)
                    gslot = 2 * g + h2
                    # HW f32->u8 conversion rounds (CoreSim truncates);
                    # +128.0 keeps the decode unbiased on HW
                    nc.vector.tensor_scalar(
                        o_t[:, 128 * gslot:128 * (gslot + 1)],
                        av2[:, 0:128], rmx[:], 128.0, ALU.mult, ALU.add)
            # natural [pix, d] layout: pix = 512w + 128*slot + p
            dd = od[i, 512 * w:512 * (w + 1), :].rearrange(
                '(g p) d -> p g d', g=4, p=128)
            nc.sync.dma_start(dd, o_t[:].rearrange('p (g x) -> p g x', g=4))
        nc.sync.dma_start(osd[i], mr_t[:])


def _legalize_dma_waits(nc):
    """walrus's PSEUDO_DMA_DIRECT2D codegen accepts at most one sync-wait
    per DMA instruction, but Tile emits 2-3 (own-lane ring wait + real
    deps).  Hoist every DMA wait onto same-engine NoOps directly before the
    DMA — engine program order makes this equivalent."""
    keep = (mybir.InstUnconditionalBranch, mybir.InstCompareAndBranch,
            mybir.InstHalt)
    f = nc.m.functions[0]
    for bb in f.blocks:
        new = []
        changed = False
        for inst in bb.instructions:
            si = inst.sync_info
            if (si is not None and len(si.on_wait) > 1
                    and not isinstance(inst, keep)):
                for wi, w in enumerate(si.on_wait[1:]):
                    new.append(mybir.InstNoOp(
                        name=f"{inst.name}-w{wi}",
                        engine=inst.engine,
                        sync_info=mybir.SyncInfo(on_wait=[w], on_update=[]),
                        bass_nofuse=True,
                    ))
                inst.sync_info = mybir.SyncInfo(
                    on_wait=[si.on_wait[0]], on_update=list(si.on_update))
                changed = True
            new.append(inst)
        if changed:
            bb.instructions = new


def _build(legalize=True, sim=False, bl=BL):
    nc = bass.Bass()
    wb = NPIX + NPIX // 8    # combined lo|hi plane width
    qlo = nc.declare_dram_parameter("qb", [bl, 128, wb], U8, isOutput=False)
    qhi = None
    klo = nc.declare_dram_parameter("kb", [bl, 128, wb], U8, isOutput=False)
    khi = None
    vlo = nc.declare_dram_parameter("vb", [bl, 128, wb], U8, isOutput=False)
    vhi = None
    sd = nc.declare_dram_parameter("scl", [128, 1], F32, isOutput=False)
    cr = nc.declare_dram_parameter("corr", [128, NT], F32, isOutput=False)
    mp = nc.declare_dram_parameter("mall", [128, 1024], F16, isOutput=False)
    idn = nc.declare_dram_parameter("idn", [128, 128], F16, isOutput=False)
    od = nc.declare_dram_parameter("out", [bl, NPIX, D], U8, isOutput=True)
    osd = nc.declare_dram_parameter("osc", [bl, 128, NT], F32, isOutput=True)
    with tile.TileContext(nc) as tc:
        with ExitStack() as ctx:
            _body(ctx, tc, qlo, qhi, klo, khi, vlo, vhi, sd, cr, mp, idn,
                  od, osd, sim=sim, bl=bl)
    if legalize:
        _legalize_dma_waits(nc)
    return nc


_CACHE = {}


def _get_nc():
    if "nc" not in _CACHE:
        _CACHE["nc"] = _build()
    return _CACHE["nc"]


_M_ALL, _CORR = _host_consts()
_IDN = np.eye(128, dtype=np.float16)


_PZ = np.empty((BL, 128, NPIX), np.float32)
_PB = np.empty((BL, 128, 8, NPIX // 8), bool)


def _pack9(x, s):
    """f32 [4,128,H,W] slice -> one [4,128,4608] u8 wire buffer.

    u9 = floor(x*s + 256.5) in [1,511]; cols 0:4096 hold u9 & 255 (the
    f32->u8 cast truncates then wraps mod 256), cols 4096:4608 pack the
    top bit of pixels j, j+512, ..., j+3584 into byte j.  Scratch buffers
    are module-level; the returned buffer is fresh (it is handed to async
    device_put).
    """
    np.multiply(x.reshape(-1, 128, NPIX), np.float32(s), out=_PZ)
    np.add(_PZ, np.float32(QOFF + 0.5), out=_PZ)
    buf = np.empty((BL, 128, NPIX + NPIX // 8), np.uint8)
    np.copyto(buf[:, :, :NPIX], _PZ, casting='unsafe')
    np.greater_equal(_PZ.reshape(-1, 128, 8, NPIX // 8),
                     np.float32(QOFF), out=_PB)
    bv = _PB.view(np.uint8)
    hb = buf[:, :, NPIX:]
    np.copyto(hb, bv[:, :, 0])
    for j in range(1, 8):
        np.left_shift(bv[:, :, j], j, out=bv[:, :, j])
        np.bitwise_or(hb, bv[:, :, j], out=hb)
    return buf


def _absmax(x):
    return float(max(x.max(), -float(x.min()), 1e-30))


def _get_runner():
    """Cached PJRT runner: one shard_map jit over 8 cores; per-call inputs
    arrive as already-committed per-device buffers assembled into global
    arrays (so transfers stream while later cores still pack)."""
    if "run" in _CACHE:
        return _CACHE["run"]
    import jax
    import jax.numpy as jnp
    from jax.sharding import Mesh, PartitionSpec, NamedSharding
    from concourse import bass2jax, mybir as mb

    nc = _get_nc()
    bass2jax.install_neuronx_cc_hook()

    part_name = nc.partition_id_tensor.name if nc.partition_id_tensor else None
    in_names, out_names, out_avals = [], [], []
    for alloc in nc.m.functions[0].allocations:
        if not isinstance(alloc, mb.MemoryLocationSet):
            continue
        name = alloc.memorylocations[0].name
        if alloc.kind == "ExternalInput":
            if name != part_name:
                in_names.append(name)
        elif alloc.kind == "ExternalOutput":
            out_names.append(name)
            out_avals.append(jax.core.ShapedArray(
                tuple(alloc.tensor_shape), mb.dt.np(alloc.dtype)))
    n_params = len(in_names)
    all_in_names = in_names + out_names
    if part_name is not None:
        all_in_names = all_in_names + [part_name]

    def _body_fn(*args):
        operands = list(args)
        if part_name is not None:
            operands.append(bass2jax.partition_id_tensor())
        outs = bass2jax._bass_exec_p.bind(
            *operands,
            out_avals=tuple(out_avals),
            in_names=tuple(all_in_names),
            out_names=tuple(out_names),
            lowering_input_output_aliases=(),
            sim_require_finite=True,
            sim_require_nnan=True,
            nc=nc,
        )
        return tuple(outs)

    devices = jax.devices()[:NCORES]
    mesh = Mesh(np.asarray(devices), ("core",))
    n_outs = len(out_names)
    in_specs = (PartitionSpec("core"),) * (n_params + n_outs)
    out_specs = (PartitionSpec("core"),) * n_outs
    from jax.experimental.shard_map import shard_map
    sharded = jax.jit(
        shard_map(_body_fn, mesh=mesh, in_specs=in_specs, out_specs=out_specs,
                  check_rep=False),
        donate_argnums=tuple(range(n_params, n_params + n_outs)),
        keep_unused=True,
    )
    sharding = NamedSharding(mesh, PartitionSpec("core"))
    zero_fns = [
        jax.jit(
            (lambda shp, dt: (lambda: jnp.zeros(shp, dt)))(
                (NCORES * av.shape[0], *av.shape[1:]), av.dtype),
            out_shardings=sharding)
        for av in out_avals
    ]

    # one-time committed constants
    dconsts = {
        "corr": jax.device_put(np.tile(_CORR, (NCORES, 1)), sharding),
        "mall": jax.device_put(np.tile(_M_ALL, (NCORES, 1)), sharding),
        "idn": jax.device_put(np.tile(_IDN, (NCORES, 1)), sharding),
    }
    for a in dconsts.values():
        a.block_until_ready()

    wb = NPIX + NPIX // 8
    gshapes = {
        "qb": (B, 128, wb), "kb": (B, 128, wb), "vb": (B, 128, wb),
    }

    def run(percore, gfeeds):
        # percore: dict name -> list of 8 per-device committed jax arrays;
        # gfeeds: dict name -> already-global committed jax arrays
        feed = dict(dconsts)
        feed.update(gfeeds)
        for name, bufs in percore.items():
            feed[name] = jax.make_array_from_single_device_arrays(
                gshapes[name], sharding, bufs)
        # donation buffers are prefetched at the end of the previous call
        zeros = _CACHE.pop("zeros_next", None) or [zf() for zf in zero_fns]
        outs = sharded(*[feed[n] for n in in_names], *zeros)
        for o in outs:
            o.copy_to_host_async()
        _CACHE["zeros_next"] = [zf() for zf in zero_fns]
        return {name: outs[i] for i, name in enumerate(out_names)}

    _CACHE["run"] = run
    _CACHE["devices"] = devices
    _CACHE["sharding"] = sharding
    return run


def _get_putter():
    if "put" not in _CACHE:
        from concurrent.futures import ThreadPoolExecutor
        _CACHE["put"] = ThreadPoolExecutor(max_workers=1)
    return _CACHE["put"]


def kernel(q, k, v):
    import jax
    run = _get_runner()          # ensures devices/consts ready
    devices = _CACHE["devices"]
    putter = _get_putter()
    q = np.asarray(q).reshape(B, D, H, W)
    k = np.asarray(k).reshape(B, D, H, W)
    v = np.asarray(v).reshape(B, D, H, W)

    futs = {n: [] for n in ("qb", "kb", "vb")}
    scl_h = np.empty((NCORES * 128, 1), np.float32)
    mv = []
    for c in range(NCORES):
        dev = devices[c]
        sl = slice(BL * c, BL * (c + 1))
        mq = _absmax(q[sl]); sq = QMAX / mq
        futs["qb"].append(putter.submit(jax.device_put, _pack9(q[sl], sq),
                                        dev))
        mk = _absmax(k[sl]); sk = QMAX / mk
        futs["kb"].append(putter.submit(jax.device_put, _pack9(k[sl], sk),
                                        dev))
        mvx = _absmax(v[sl]); sv = QMAX / mvx
        futs["vb"].append(putter.submit(jax.device_put, _pack9(v[sl], sv),
                                        dev))
        mv.append(mvx)
        scl_h[128 * c:128 * (c + 1)] = SCALE / (sq * sk)
    scl_f = putter.submit(jax.device_put, scl_h, _CACHE["sharding"])

    percore = {n: [f.result() for f in fs] for n, fs in futs.items()}
    outs = run(percore, {"scl": scl_f.result()})
    out_g = outs["out"]           # [B, NPIX, D] u8, 8-core sharded
    osc_g = outs["osc"]           # [B, 128, NT] f32, mr = mx*G/denom

    res = np.empty((B, NPIX, D), np.float32)
    shards = {int(s.index[0].start) // BL: s
              for s in out_g.addressable_shards}
    oshards = {int(s.index[0].start) // BL: s
               for s in osc_g.addressable_shards}
    for c in range(NCORES):
        oc = np.asarray(shards[c].data)    # [BL, NPIX, D] u8
        mr = np.asarray(oshards[c].data)   # [BL, 128, NT] f32
        # out = (u8-128) * mr * Mv / (OMAX * G * QMAX), scale per (i,t,p)
        wgt = mr.transpose(0, 2, 1) * np.float32(mv[c] / (OMAX * G * QMAX))
        rs = res[BL * c:BL * (c + 1)]
        np.subtract(oc.astype(np.float32), np.float32(128.0), out=rs)
        np.multiply(rs.reshape(BL, NT, 128, D), wgt[:, :, :, None], out=rs.reshape(BL, NT, 128, D))
    return res.reshape(B, H, W, D)


def _profile(inputs):
    """Run once under the axon NTFF profiling hook; returns max-core HW
    exec time in ns (None if the hook is unavailable). For test.py only."""
    from concourse.bass_utils import run_bass_kernel_spmd
    nc = _get_nc()
    q = np.asarray(inputs["q"]).reshape(B, D, H, W)
    k = np.asarray(inputs["k"]).reshape(B, D, H, W)
    v = np.asarray(inputs["v"]).reshape(B, D, H, W)
    in_maps = []
    for c in range(NCORES):
        sl = slice(BL * c, BL * (c + 1))
        mq = _absmax(q[sl]); sq = QMAX / mq
        mk = _absmax(k[sl]); sk = QMAX / mk
        mvx = _absmax(v[sl]); sv = QMAX / mvx
        in_maps.append({
            "qb": _pack9(q[sl], sq), "kb": _pack9(k[sl], sk),
            "vb": _pack9(v[sl], sv),
            "scl": np.full((128, 1), SCALE / (sq * sk), np.float32),
            "corr": _CORR, "mall": _M_ALL, "idn": _IDN,
        })
    br = run_bass_kernel_spmd(nc, in_maps, core_ids=list(range(NCORES)),
                              trace=True)
    res = {
        "exec_time_ns": br.exec_time_ns,
        "mean_exec_time_ns": br.mean_exec_time_ns,
        "trace": br.instructions_and_trace,
    }
    _CACHE["last_profile"] = res
    if br.instructions_and_trace is not None:
        print("trace path:", br.instructions_and_trace[1])
    return br.exec_time_ns
